# revision 24
# baseline (speedup 1.0000x reference)
"""AttrFusion Trainium2 kernel — 8-core SPMD.

Sharding:
  scales 0-2 (H=128/64/32): spatial row-sharding, each core computes all 8
    classes for its H/8 row slice; class-sum + residual + relu on device.
  scales 3-4 (H=16/8): class-sharding, core c computes class c full-spatial;
    the 8-way class sum + residual + relu happen on host (tiny arrays).

Per-class conv = 9-tap matmul accumulation in PSUM:
  out[cout, pix] += W[tap][cin,cout].T @ slab[cin, pix_shifted]
with bf16 inputs, fp32 PSUM accumulation. BN affine and the softmax attention
weight fold into a single per-channel scale/bias applied by the ACT engine
together with the inner relu: attn>0 so attn*relu(y*s+t) = relu(y*s*attn+t*attn).
"""
import numpy as np
import ml_dtypes

import concourse.bacc as bacc
import concourse.mybir as mybir
from concourse.tile import TileContext
from concourse.bass_utils import run_bass_kernel_spmd

BF16 = mybir.dt.bfloat16
F32 = mybir.dt.float32
NBF = ml_dtypes.bfloat16

B = 2
K = 8
C = 256
NCORES = 8
EPS = 1e-5
SRC = 64  # attr_feat spatial size
# (H, W) per scale; scales 0-2 row-sharded, 3-4 class-sharded
SHAPES = [(128, 128), (64, 64), (32, 32), (16, 16), (8, 8)]
SPATIAL = [0, 1, 2]
CLASSSH = [3, 4]
RPC = {i: SHAPES[i][0] // NCORES for i in SPATIAL}  # rows per core: 16, 8, 4
# N-tile rows per matmul for spatial scales (nr*W <= 512)
NR = {0: 4, 1: 8, 2: 4}

LAST_RESULTS = None  # BassKernelResults of the most recent run (for test.py)
_NC_CACHE = None

# schedule tuning knobs
SLAB_BUFS = 2
WT_BUFS = 2
SPLIT_SLAB_DMA = 1  # 1: one DMA per slab tile; 2: per-batch; 4: per-batch half-rows
SCALE_ORDER = [0, 1, 2, 3, 4]
# 1-D Winograd F(2,3) along x for scale 0: 24 MMs per 1024 output pixels
# instead of 36 (1.5x fewer PE columns). Input transform (+-1 coeffs) and the
# A^T output combine run on DVE, which has slack.
WINOGRAD_S0 = True
# s1's N=256 tiles make the A^T epilogue overhead exceed the 39us MM saving
# (sim: 610us vs 600us) — keep direct conv there.
WINOGRAD_S1 = False


def _col_A(i, b, k, m):
    return ((i * 2 + b) * 8 + k) * 2 + m


def _col_B(i, b, m):  # i in {0,1} for scales 3,4
    return 96 + (i * 2 + b) * 2 + m


NPARAM = 104


def _build_nc():
    nc = bacc.Bacc()

    # ---- DRAM I/O ----
    slabs = []
    for i in SPATIAL:
        H, W = SHAPES[i]
        t = nc.dram_tensor(f"slab{i}", [16, 128, B, RPC[i] + 2, W + 2], BF16,
                           kind="ExternalInput")
        slabs.append(t)
    for i in CLASSSH:
        H, W = SHAPES[i]
        t = nc.dram_tensor(f"slab{i}", [2, 128, B, H + 2, W + 2], BF16,
                           kind="ExternalInput")
        slabs.append(t)
    # weights: wA for scales 0-2 (all classes), wB for scales 3-4 (own class)
    wA = nc.dram_tensor("wA", [3, K, 2, 128, 2, 9, 128], BF16, kind="ExternalInput")
    # x-Winograd weights for scale 0: U[u] = sum_tx G[u,tx] w[.,.,ty,tx]
    wG = nc.dram_tensor("wG", [K, 2, 128, 2, 3, 4, 128], BF16, kind="ExternalInput")
    wH = nc.dram_tensor("wH", [K, 2, 128, 2, 3, 4, 128], BF16, kind="ExternalInput")
    wB = nc.dram_tensor("wB", [2, 2, 128, 2, 9, 128], BF16, kind="ExternalInput")
    sp_d = nc.dram_tensor("sp", [128, NPARAM], F32, kind="ExternalInput")
    tp_d = nc.dram_tensor("tp", [128, NPARAM], F32, kind="ExternalInput")
    feats = [nc.dram_tensor(f"f{i}", [2, 128, B, RPC[i], SHAPES[i][1]], F32,
                            kind="ExternalInput") for i in SPATIAL]
    outs = []
    for i in SPATIAL:
        outs.append(nc.dram_tensor(f"o{i}", [2, 128, B, RPC[i], SHAPES[i][1]], F32,
                                   kind="ExternalOutput"))
    for i in CLASSSH:
        H, W = SHAPES[i]
        outs.append(nc.dram_tensor(f"o{i}", [2, 128, B, H, W], F32,
                                   kind="ExternalOutput"))

    A = mybir.ActivationFunctionType

    with TileContext(nc) as tc:
        with (
            tc.tile_pool(name="const", bufs=1) as cpool,
            tc.tile_pool(name="slab", bufs=SLAB_BUFS) as spool,
            tc.tile_pool(name="wt", bufs=WT_BUFS) as wpool,
            tc.tile_pool(name="acc", bufs=2) as apool,
            tc.tile_pool(name="tmp", bufs=4) as tpool,
            tc.tile_pool(name="ps", bufs=8, space="PSUM") as ppool,
        ):
            spt = cpool.tile([128, NPARAM], F32)
            tpt = cpool.tile([128, NPARAM], F32)
            nc.sync.dma_start(out=spt, in_=sp_d[:, :])
            nc.sync.dma_start(out=tpt, in_=tp_d[:, :])

            # ---------- spatial-sharded scales ----------
            def emit_spatial(i):
                H, W = SHAPES[i]
                rpc = RPC[i]
                nr = NR[i]
                # accumulators initialized with the residual feat slice
                acc0 = apool.tile([128, B, rpc, W], F32, tag="acc0")
                acc1 = apool.tile([128, B, rpc, W], F32, tag="acc1")
                accs = [acc0, acc1]
                nc.sync.dma_start(out=acc0, in_=feats[i][0])
                nc.sync.dma_start(out=acc1, in_=feats[i][1])

                # N-tile groups: list of lists of (b, r0)
                tiles = [(b, r0) for b in range(B) for r0 in range(0, rpc, nr)]
                gsz = 4 if i == 0 else 2
                groups = [tiles[g:g + gsz] for g in range(0, len(tiles), gsz)]

                for k in range(K):
                    sl0 = spool.tile([128, B, rpc + 2, W + 2], BF16, tag="sl0")
                    sl1 = spool.tile([128, B, rpc + 2, W + 2], BF16, tag="sl1")
                    # The very first slab load gates the kernel's first matmul
                    # burst; split it finely so it streams on many DMA queues
                    # in parallel. Steady-state classes use one DMA per tile
                    # (splitting there measurably hurts: extra sem overhead).
                    first = (k == 0 and i == SCALE_ORDER[0])
                    for sl, src in ((sl0, slabs[i][2 * k]), (sl1, slabs[i][2 * k + 1])):
                        if first:
                            nchunk = 3
                            step = (rpc + 2 + nchunk - 1) // nchunk
                            nq = 0
                            for b in range(B):
                                for r0 in range(0, rpc + 2, step):
                                    r1 = min(r0 + step, rpc + 2)
                                    eng = nc.sync if nq % 2 == 0 else nc.gpsimd
                                    eng.dma_start(out=sl[:, b, r0:r1],
                                                  in_=src[:, b, r0:r1])
                                    nq += 1
                        else:
                            nc.sync.dma_start(out=sl, in_=src)
                    sls = [sl0, sl1]
                    wt0 = wpool.tile([128, 2, 9, 128], BF16, tag="wt0")
                    wt1 = wpool.tile([128, 2, 9, 128], BF16, tag="wt1")
                    nc.sync.dma_start(out=wt0, in_=wA[i, k, 0])
                    nc.sync.dma_start(out=wt1, in_=wA[i, k, 1])
                    wts = [wt0, wt1]
                    for m in range(2):
                        for grp in groups:
                            pss = [ppool.tile([128, nr, W], F32, tag="ps", name="ps")
                                   for _ in grp]
                            for ci in range(2):
                                for tap in range(9):
                                    dy, dx = tap // 3, tap % 3
                                    lhsT = wts[ci][:, m, tap, :]
                                    for ps, (b, r0) in zip(pss, grp):
                                        rhs = sls[ci][:, b, r0 + dy:r0 + dy + nr,
                                                      dx:dx + W]
                                        nc.tensor.matmul(
                                            ps, lhsT, rhs,
                                            start=(ci == 0 and tap == 0),
                                            stop=(ci == 1 and tap == 8))
                            for ps, (b, r0) in zip(pss, grp):
                                col = _col_A(i, b, k, m)
                                tmp = tpool.tile([128, nr, W], F32, tag="tmp", bufs=3)
                                nc.scalar.activation(
                                    tmp, ps, A.Relu,
                                    bias=tpt[:, col:col + 1],
                                    scale=spt[:, col:col + 1])
                                dst = accs[m][:, b, r0:r0 + nr, :]
                                nc.vector.tensor_add(dst, dst, tmp)
                for m in range(2):
                    nc.vector.tensor_scalar_max(accs[m], accs[m], 0.0)
                    nc.sync.dma_start(out=outs[i][m], in_=accs[m])

            def emit_winograd(i, wg_dram):
                H, W = SHAPES[i]
                rpc = RPC[i]
                XT = W // 2           # winograd output pairs per row
                nr = min(rpc, 8)      # rows per psum tile
                acc0 = apool.tile([128, B, rpc, W], F32, tag="acc0")
                acc1 = apool.tile([128, B, rpc, W], F32, tag="acc1")
                accs = [acc0, acc1]
                nc.sync.dma_start(out=acc0, in_=feats[i][0])
                nc.sync.dma_start(out=acc1, in_=feats[i][1])

                for k in range(K):
                    sl0 = spool.tile([128, B, rpc + 2, W + 2], BF16, tag="sl0")
                    sl1 = spool.tile([128, B, rpc + 2, W + 2], BF16, tag="sl1")
                    first = (k == 0 and SCALE_ORDER[0] == 0)
                    for sl, src in ((sl0, slabs[i][2 * k]), (sl1, slabs[i][2 * k + 1])):
                        if first:
                            step = 6
                            nq = 0
                            for b in range(B):
                                for r0 in range(0, rpc + 2, step):
                                    r1 = min(r0 + step, rpc + 2)
                                    eng = nc.sync if nq % 2 == 0 else nc.gpsimd
                                    eng.dma_start(out=sl[:, b, r0:r1],
                                                  in_=src[:, b, r0:r1])
                                    nq += 1
                        else:
                            nc.sync.dma_start(out=sl, in_=src)
                    sls = [sl0, sl1]
                    wg0 = wpool.tile([128, 2, 3, 4, 128], BF16, tag="wg0", bufs=2)
                    wg1 = wpool.tile([128, 2, 3, 4, 128], BF16, tag="wg1", bufs=2)
                    nc.sync.dma_start(out=wg0, in_=wg_dram[k, 0])
                    nc.sync.dma_start(out=wg1, in_=wg_dram[k, 1])
                    wgs = [wg0, wg1]
                    for b in range(B):
                        # input transform: V[u] over x windows (stride 2)
                        vt0 = tpool.tile([128, rpc + 2, 4, XT], BF16, tag="vt0", bufs=2)
                        vt1 = tpool.tile([128, rpc + 2, 4, XT], BF16, tag="vt1", bufs=2)
                        vts = [vt0, vt1]
                        for ci, (sl, vt) in enumerate(zip(sls, vts)):
                            d0 = sl[:, b, :, 0:W:2]
                            d1 = sl[:, b, :, 1:W + 1:2]
                            d2 = sl[:, b, :, 2:W + 2:2]
                            d3 = sl[:, b, :, 3:W + 2:2]
                            nc.vector.tensor_sub(vt[:, :, 0, :], d0, d2)
                            nc.vector.tensor_add(vt[:, :, 1, :], d1, d2)
                            nc.vector.tensor_sub(vt[:, :, 2, :], d2, d1)
                            nc.vector.tensor_sub(vt[:, :, 3, :], d1, d3)
                        for m in range(2):
                            for nt in range(rpc // nr):
                                r0 = nt * nr
                                pss = [ppool.tile([128, nr, XT], F32, tag="ps",
                                                  name="ps") for _ in range(4)]
                                for ci in range(2):
                                    for ty in range(3):
                                        for u in range(4):
                                            lhsT = wgs[ci][:, m, ty, u, :]
                                            rhs = vts[ci][:, r0 + ty:r0 + ty + nr,
                                                          u, :]
                                            nc.tensor.matmul(
                                                pss[u], lhsT, rhs,
                                                start=(ci == 0 and ty == 0),
                                                stop=(ci == 1 and ty == 2))
                                # A^T combine: y0 = m0+m1+m2, y1 = m1-m2-m3.
                                # DVE may read only one PSUM operand per op, so
                                # stage m1 into SBUF via an ACT copy first.
                                c1 = tpool.tile([128, nr, XT], F32, tag="c1", bufs=2)
                                nc.scalar.copy(c1, pss[1])
                                t0 = tpool.tile([128, nr, XT], F32, tag="t0", bufs=2)
                                t1 = tpool.tile([128, nr, XT], F32, tag="t1", bufs=2)
                                nc.vector.tensor_add(t0, c1, pss[0])
                                nc.vector.tensor_add(t0, t0, pss[2])
                                nc.vector.tensor_sub(t1, c1, pss[2])
                                nc.vector.tensor_sub(t1, t1, pss[3])
                                col = _col_A(i, b, k, m)
                                y0 = tpool.tile([128, nr, XT], BF16, tag="y0", bufs=2)
                                y1 = tpool.tile([128, nr, XT], BF16, tag="y1", bufs=2)
                                nc.scalar.activation(
                                    y0, t0, A.Relu,
                                    bias=tpt[:, col:col + 1],
                                    scale=spt[:, col:col + 1])
                                nc.scalar.activation(
                                    y1, t1, A.Relu,
                                    bias=tpt[:, col:col + 1],
                                    scale=spt[:, col:col + 1])
                                dste = accs[m][:, b, r0:r0 + nr, 0:W:2]
                                dsto = accs[m][:, b, r0:r0 + nr, 1:W:2]
                                nc.vector.tensor_add(dste, dste, y0)
                                nc.vector.tensor_add(dsto, dsto, y1)
                for m in range(2):
                    nc.vector.tensor_scalar_max(accs[m], accs[m], 0.0)
                    nc.sync.dma_start(out=outs[i][m], in_=accs[m])

            # ---------- class-sharded scales (3, 4) ----------
            def emit_classsh(i):
                j = CLASSSH.index(i)
                H, W = SHAPES[i]
                sl0 = spool.tile([128, B, H + 2, W + 2], BF16, tag="sl0")
                sl1 = spool.tile([128, B, H + 2, W + 2], BF16, tag="sl1")
                nc.sync.dma_start(out=sl0, in_=slabs[i][0])
                nc.sync.dma_start(out=sl1, in_=slabs[i][1])
                sls = [sl0, sl1]
                wt0 = wpool.tile([128, 2, 9, 128], BF16, tag="wt0")
                wt1 = wpool.tile([128, 2, 9, 128], BF16, tag="wt1")
                nc.sync.dma_start(out=wt0, in_=wB[j, 0])
                nc.sync.dma_start(out=wt1, in_=wB[j, 1])
                wts = [wt0, wt1]
                for m in range(2):
                    stg = tpool.tile([128, B, H, W], F32, tag="stg", bufs=2)
                    pss = [ppool.tile([128, H, W], F32, tag="ps", name="ps") for _ in range(B)]
                    for ci in range(2):
                        for tap in range(9):
                            dy, dx = tap // 3, tap % 3
                            lhsT = wts[ci][:, m, tap, :]
                            for b, ps in enumerate(pss):
                                rhs = sls[ci][:, b, dy:dy + H, dx:dx + W]
                                nc.tensor.matmul(
                                    ps, lhsT, rhs,
                                    start=(ci == 0 and tap == 0),
                                    stop=(ci == 1 and tap == 8))
                    for b, ps in enumerate(pss):
                        col = _col_B(j, b, m)
                        nc.scalar.activation(
                            stg[:, b], ps, A.Relu,
                            bias=tpt[:, col:col + 1],
                            scale=spt[:, col:col + 1])
                    nc.sync.dma_start(out=outs[i][m], in_=stg)

            for i in SCALE_ORDER:
                if i == 0 and WINOGRAD_S0:
                    emit_winograd(0, wG)
                elif i == 1 and WINOGRAD_S1:
                    emit_winograd(1, wH)
                elif i in SPATIAL:
                    emit_spatial(i)
                else:
                    emit_classsh(i)

    nc.compile()
    return nc


def _resize_ac(x, H, W):
    """Bilinear align_corners=True resize, numpy fp32. x: [B,Ch,h,w]."""
    h, w = x.shape[2], x.shape[3]
    if (h, w) == (H, W):
        return x
    ys = np.linspace(0.0, h - 1.0, H)
    xs = np.linspace(0.0, w - 1.0, W)
    y0 = np.floor(ys).astype(np.int64)
    x0 = np.floor(xs).astype(np.int64)
    y1 = np.minimum(y0 + 1, h - 1)
    x1 = np.minimum(x0 + 1, w - 1)
    wy = (ys - y0).astype(np.float32)[None, None, :, None]
    wx = (xs - x0).astype(np.float32)[None, None, None, :]
    rows = x[:, :, y0, :] * (1.0 - wy) + x[:, :, y1, :] * wy
    return rows[:, :, :, x0] * (1.0 - wx) + rows[:, :, :, x1] * wx


def _softmax(z):
    z = z - z.max(axis=-1, keepdims=True)
    e = np.exp(z)
    return e / e.sum(axis=-1, keepdims=True)


def kernel(feat0, feat1, feat2, feat3, feat4, attr_feat, mlp_f_w, mlp_f_b,
           mlp_a_w, mlp_a_b, conv_w, bn_gamma, bn_beta, bn_mean, bn_var):
    global LAST_RESULTS, _NC_CACHE
    feats_np = [np.asarray(f, dtype=np.float32)
                for f in (feat0, feat1, feat2, feat3, feat4)]
    attr = np.asarray(attr_feat, dtype=np.float32)
    conv_w = np.asarray(conv_w, dtype=np.float32)

    # ---- attention weights (host; ~0.1% of FLOPs) ----
    avg_attr = attr.mean(axis=(2, 3)).reshape(B, K, C)
    avg_attr = avg_attr @ np.asarray(mlp_a_w).T + np.asarray(mlp_a_b)
    attn = []
    for f in feats_np:
        avg_x = f.mean(axis=(2, 3)) @ np.asarray(mlp_f_w).T + np.asarray(mlp_f_b)
        logits = np.einsum('bkc,bc->bk', avg_attr, avg_x)
        attn.append(_softmax(logits).astype(np.float32))

    # ---- BN fold ----
    inv = 1.0 / np.sqrt(np.asarray(bn_var) + EPS)
    s_all = (np.asarray(bn_gamma) * inv).astype(np.float32)          # [5, 2048]
    t_all = (np.asarray(bn_beta) - np.asarray(bn_mean) * s_all).astype(np.float32)

    # ---- params tensor (per-core: differs only in the class-sharded cols) ----
    sp = np.zeros((NCORES, 128, NPARAM), dtype=np.float32)
    tp = np.zeros((NCORES, 128, NPARAM), dtype=np.float32)
    for i in SPATIAL:
        for b in range(B):
            for k in range(K):
                for m in range(2):
                    col = _col_A(i, b, k, m)
                    ch = slice(k * C + m * 128, k * C + (m + 1) * 128)
                    sp[:, :, col] = s_all[i, ch] * attn[i][b, k]
                    tp[:, :, col] = t_all[i, ch] * attn[i][b, k]
    for j, i in enumerate(CLASSSH):
        for b in range(B):
            for m in range(2):
                col = _col_B(j, b, m)
                for c in range(NCORES):
                    ch = slice(c * C + m * 128, c * C + (m + 1) * 128)
                    sp[c, :, col] = s_all[i, ch] * attn[i][b, c]
                    tp[c, :, col] = t_all[i, ch] * attn[i][b, c]

    # ---- resize + pad + bf16 slabs ----
    slab_arrays = {}  # name -> list per core
    for idx, (H, W) in enumerate(SHAPES):
        rs = _resize_ac(attr, H, W)
        pad = np.zeros((B, K * C, H + 2, W + 2), dtype=np.float32)
        pad[:, :, 1:-1, 1:-1] = rs
        pad = pad.astype(NBF)
        if idx in SPATIAL:
            rpc = RPC[idx]
            per_core = []
            for c in range(NCORES):
                sl = pad[:, :, c * rpc: c * rpc + rpc + 2, :]
                sl = np.ascontiguousarray(
                    sl.reshape(B, 16, 128, rpc + 2, W + 2).transpose(1, 2, 0, 3, 4))
                per_core.append(sl)
        else:
            per_core = []
            for c in range(NCORES):
                sl = pad[:, c * C:(c + 1) * C]
                sl = np.ascontiguousarray(
                    sl.reshape(B, 2, 128, H + 2, W + 2).transpose(1, 2, 0, 3, 4))
                per_core.append(sl)
        slab_arrays[f"slab{idx}"] = per_core

    # ---- weights ----
    # conv_w[i]: [2048, 256, 3, 3] -> [k, cinT, cin, coutT, tap, cout]
    wtr = []
    for i in range(5):
        v = conv_w[i].reshape(K, 2, 128, 2, 128, 3, 3)  # k,coutT,cout,cinT,cin,ty,tx
        v = v.transpose(0, 3, 4, 1, 5, 6, 2).reshape(K, 2, 128, 2, 9, 128)
        wtr.append(v.astype(NBF))
    wA_arr = np.ascontiguousarray(np.stack(wtr[:3]))          # [3,K,2,128,2,9,128]
    # x-Winograd weights for scale 0: G rows u0=w0, u1=(w0+w1+w2)/2,
    # u2=(w0-w1+w2)/2, u3=w2 applied along tx
    g = np.array([[1, 0, 0], [.5, .5, .5], [.5, -.5, .5], [0, 0, 1]],
                 dtype=np.float32)
    def _wino_weights(wi):
        v = wi.reshape(K, 2, 128, 2, 128, 3, 3)      # k,mT,cout,ciT,cin,ty,tx
        w = np.tensordot(v, g, axes=([6], [1]))      # k,mT,cout,ciT,cin,ty,u
        w = w.transpose(0, 3, 4, 1, 5, 6, 2)         # k,ciT,cin,mT,ty,u,cout
        return np.ascontiguousarray(w).astype(NBF)   # [K,2,128,2,3,4,128]
    wG_arr = _wino_weights(conv_w[0])
    wH_arr = _wino_weights(conv_w[1])
    wB_per_core = [np.ascontiguousarray(np.stack([wtr[3][c], wtr[4][c]]))
                   for c in range(NCORES)]                     # [2,2,128,2,9,128]

    # ---- residual feat slices ----
    feat_slices = {}
    for i in SPATIAL:
        H, W = SHAPES[i]
        rpc = RPC[i]
        per_core = []
        for c in range(NCORES):
            fs = feats_np[i][:, :, c * rpc:(c + 1) * rpc, :]
            fs = np.ascontiguousarray(
                fs.reshape(B, 2, 128, rpc, W).transpose(1, 2, 0, 3, 4))
            per_core.append(fs)
        feat_slices[f"f{i}"] = per_core

    in_maps = []
    for c in range(NCORES):
        m = {f"slab{i}": slab_arrays[f"slab{i}"][c] for i in range(5)}
        m["wA"] = wA_arr
        m["wG"] = wG_arr
        m["wH"] = wH_arr
        m["wB"] = wB_per_core[c]
        m["sp"] = np.ascontiguousarray(sp[c])
        m["tp"] = np.ascontiguousarray(tp[c])
        for i in SPATIAL:
            m[f"f{i}"] = feat_slices[f"f{i}"][c]
        in_maps.append(m)

    if _NC_CACHE is None:
        _NC_CACHE = _build_nc()
    nc = _NC_CACHE

    try:
        res = run_bass_kernel_spmd(nc, in_maps, core_ids=list(range(NCORES)))
    except Exception:
        # transient axon-tunnel hiccups (e.g. mesh desync) are retryable
        res = run_bass_kernel_spmd(nc, in_maps, core_ids=list(range(NCORES)))
    LAST_RESULTS = res

    # ---- assemble outputs ----
    outputs = []
    for i in SPATIAL:
        H, W = SHAPES[i]
        rpc = RPC[i]
        st = np.stack([res.results[c][f"o{i}"] for c in range(NCORES)])
        # [core, cT, 128, b, rpc, W] -> [b, cT, 128, core, rpc, W]
        full = st.transpose(3, 1, 2, 0, 4, 5).reshape(B, C, H, W)
        outputs.append(full)
    for i in CLASSSH:
        H, W = SHAPES[i]
        z = np.zeros((2, 128, B, H, W), dtype=np.float32)
        for c in range(NCORES):
            z += res.results[c][f"o{i}"]
        z = z.transpose(2, 0, 1, 3, 4).reshape(B, C, H, W)
        outputs.append(np.maximum(feats_np[i] + z, 0.0))
    return tuple(outputs)
